# revision 4
# baseline (speedup 1.0000x reference)
"""Local 9x9 correlation (cost volume) kernel for Trainium2.

out[b, di*9+dj, h, w] = (1/C) * sum_c x1[b,c,h,w] * x2pad[b,c,h+di,w+dj]

Strategy: batch-parallel across 8 NeuronCores (1 sample each). On-core, for
each output row h the correlation is a banded Gram matrix between the x1 row
(stationary, 4 col-tiled strips of 32 positions) and a 9-row window of the
zero-padded x2. The moving operand streams in (n, di) order (window column
outer, row displacement inner), so each output partition m's 81 useful values
land in ONE contiguous 162 B slice of the PSUM band at offset 9*m. The band
is evacuated to SBUF in bf16 and dumped with 16 compact sub-DMAs per chunk
(one per 8-partition group, 144-col window) - 2.5x less HBM write traffic
than dumping the full band. x2 chunk halos come from SBUF-SBUF copies of the
previous chunk instead of HBM re-reads.
"""

import numpy as np

B, C, H, W = 8, 256, 96, 128
R = 4                 # correlation radius
D = 2 * R + 1         # 9 displacements per axis
HCHUNK = 32
NCHUNK = H // HCHUNK  # 3
STRIP = 32            # x1 positions per PE column-group
NSTRIP = W // STRIP   # 4
WIN = STRIP + 2 * R   # 40 moving columns per strip
BAND = D * WIN        # 360 PSUM band columns per strip
PADW = W + 2 * R      # 136
PADROWS = HCHUNK + 2 * R  # 40
GRP = 8               # partitions per compact dump group
NGRP = 128 // GRP     # 16
GWIN = D * (GRP - 1) + D * D  # 144 band cols covering one group's diagonals

_compiled = None
last_results = None  # BassKernelResults of the most recent run (for profiling)


def _build(reps: int = 1):
    import contextlib

    import concourse.bass as bass  # noqa: F401
    import concourse.tile as tile
    from concourse import bacc, mybir

    nc = bacc.Bacc(
        "TRN2", target_bir_lowering=False, debug=False, num_devices=8
    )
    x1 = nc.dram_tensor("x1", [C, H, W], mybir.dt.float32, kind="ExternalInput").ap()
    x2 = nc.dram_tensor("x2", [C, H, W], mybir.dt.float32, kind="ExternalInput").ap()
    dump = nc.dram_tensor(
        "dump", [W, H, GWIN], mybir.dt.bfloat16, kind="ExternalOutput"
    ).ap()

    with tile.TileContext(nc) as tc:
        with (
            tc.tile_pool(name="x1p", bufs=2) as x1p,
            tc.tile_pool(name="x2p", bufs=2) as x2p,
            tc.tile_pool(name="stg", bufs=2) as stg,
            tc.tile_pool(name="ps", bufs=4, space="PSUM") as psp,
            tc.For_i(0, reps, 1) if reps > 1 else contextlib.nullcontext(),
        ):
            prev_x2c = None
            for k in range(NCHUNK):
                h0 = k * HCHUNK

                x1c = x1p.tile([128, 2, HCHUNK, W], mybir.dt.bfloat16)
                for cc in range(2):
                    nc.gpsimd.dma_start(
                        out=x1c[:, cc, :, :],
                        in_=x1[cc * 128 : (cc + 1) * 128, h0 : h0 + HCHUNK, :],
                    )

                # padded x2 slab: local row p corresponds to x2 row h0-R+p
                x2c = x2p.tile([128, 2, PADROWS, PADW], mybir.dt.bfloat16)
                src_r0 = h0 - R
                nc.vector.memset(x2c[:, :, :, 0:R], 0.0)
                nc.vector.memset(x2c[:, :, :, PADW - R : PADW], 0.0)
                if k == 0:
                    # rows [0, R) are above the image: zero them
                    lo = R
                    nc.vector.memset(x2c[:, :, 0:R, 0:PADW], 0.0)
                else:
                    # halo rows [0, 2R) = previous chunk's local rows
                    # [HCHUNK, HCHUNK+2R) - SBUF->SBUF copy, no HBM re-read
                    lo = 2 * R
                    nc.sync.dma_start(
                        out=x2c[:, :, 0 : 2 * R, :],
                        in_=prev_x2c[:, :, HCHUNK : HCHUNK + 2 * R, :],
                    )
                hi = min(PADROWS, H - src_r0)
                if hi < PADROWS:
                    nc.vector.memset(x2c[:, :, hi:PADROWS, 0:PADW], 0.0)
                for cc in range(2):
                    nc.gpsimd.dma_start(
                        out=x2c[:, cc, lo:hi, R : R + W],
                        in_=x2[
                            cc * 128 : (cc + 1) * 128, src_r0 + lo : src_r0 + hi, :
                        ],
                    )
                prev_x2c = x2c

                stage = stg.tile([128, HCHUNK, BAND], mybir.dt.bfloat16)
                for hl in range(HCHUNK):
                    # full 2 KB bank per tile so matmul outputs stay in-bank
                    psb = psp.tile([128, 512], mybir.dt.float32)
                    ps = psb[:, 0:BAND]
                    for cc in range(2):
                        for j in range(NSTRIP):
                            # moving operand streamed (n, di): out col = n*9+di
                            nc.tensor.matmul(
                                out=ps[STRIP * j : STRIP * (j + 1), :],
                                lhsT=x1c[:, cc, hl, STRIP * j : STRIP * (j + 1)],
                                rhs=x2c[
                                    :, cc, hl : hl + D,
                                    STRIP * j : STRIP * j + WIN,
                                ].transpose([0, 2, 1]),
                                start=(cc == 0),
                                stop=(cc == 1),
                                tile_position=(0, STRIP * j),
                                skip_group_check=True,
                            )
                    if hl % 2 == 0:
                        nc.vector.tensor_scalar_mul(stage[:, hl, :], ps[:, :], 1.0)
                    else:
                        nc.scalar.mul(stage[:, hl, :], ps[:, :], 1.0)

                # compact dump: group g (partitions 8g..8g+8) only needs band
                # cols [72*(g%4), 72*(g%4)+144) - partition m's 81 values sit
                # at cols [9*(m%32), 9*(m%32)+81) within its strip's band
                for g in range(NGRP):
                    c0 = 9 * GRP * (g % (STRIP // GRP))
                    nc.sync.dma_start(
                        out=dump[GRP * g : GRP * (g + 1), h0 : h0 + HCHUNK, :],
                        in_=stage[GRP * g : GRP * (g + 1), :, c0 : c0 + GWIN],
                    )

    nc.compile()
    return nc


def _deskew(dump_b: np.ndarray) -> np.ndarray:
    """[W, H, GWIN] bf16 compact band dump -> [81, H, W] fp32.

    dump[w, h, 9*(w%8) + dj*9 + di] = corr[di, dj, h, w] (unscaled).
    """
    d = np.asarray(dump_b).astype(np.float32) * np.float32(1.0 / C)
    d = d.reshape(NGRP, GRP, H, GWIN)
    # blocks[g, m, h, dj*9+di] for w = 8g+m
    blocks = np.stack([d[:, m, :, 9 * m : 9 * m + 81] for m in range(GRP)], axis=1)
    out = blocks.reshape(NGRP, GRP, H, D, D)      # [g, m, h, dj, di]
    out = out.transpose(4, 3, 2, 0, 1)            # [di, dj, h, g, m]
    return out.reshape(D * D, H, W)


def kernel(x1: np.ndarray, x2: np.ndarray) -> np.ndarray:
    global _compiled, last_results
    import os

    os.environ["BASS_NEVER_TRACE"] = "1"
    from concourse.bass_utils import run_bass_kernel_spmd

    x1 = np.ascontiguousarray(np.asarray(x1), dtype=np.float32)
    x2 = np.ascontiguousarray(np.asarray(x2), dtype=np.float32)
    assert x1.shape == (B, C, H, W) and x2.shape == (B, C, H, W)

    if _compiled is None:
        _compiled = _build()
    nc = _compiled

    in_maps = [{"x1": x1[b], "x2": x2[b]} for b in range(B)]
    res = run_bass_kernel_spmd(nc, in_maps, core_ids=list(range(B)))
    last_results = res

    return np.stack([_deskew(res.results[b]["dump"]) for b in range(B)], axis=0)


def _timed_run(nc, x1, x2, iters):
    import time

    import jax
    from jax.experimental.shard_map import shard_map
    from jax.sharding import Mesh, PartitionSpec

    from concourse import bass2jax, mybir

    bass2jax.install_neuronx_cc_hook()

    partition_name = (
        nc.partition_id_tensor.name if nc.partition_id_tensor else None
    )
    in_names, out_names, out_avals, zeros = [], [], [], []
    for alloc in nc.m.functions[0].allocations:
        if not isinstance(alloc, mybir.MemoryLocationSet):
            continue
        name = alloc.memorylocations[0].name
        if alloc.kind == "ExternalInput":
            if name != partition_name:
                in_names.append(name)
        elif alloc.kind == "ExternalOutput":
            shape = tuple(alloc.tensor_shape)
            dtype = mybir.dt.np(alloc.dtype)
            out_names.append(name)
            out_avals.append(jax.core.ShapedArray(shape, dtype))
            zeros.append(np.zeros(shape, dtype))
    n_params = len(in_names)
    all_names = in_names + out_names
    if partition_name is not None:
        all_names = all_names + [partition_name]

    def _body(*args):
        operands = list(args)
        if partition_name is not None:
            operands.append(bass2jax.partition_id_tensor())
        return tuple(
            bass2jax._bass_exec_p.bind(
                *operands,
                out_avals=tuple(out_avals),
                in_names=tuple(all_names),
                out_names=tuple(out_names),
                lowering_input_output_aliases=(),
                sim_require_finite=True,
                sim_require_nnan=True,
                nc=nc,
            )
        )

    devices = jax.devices()[:B]
    mesh = Mesh(np.asarray(devices), ("core",))
    specs = (PartitionSpec("core"),) * (n_params + len(out_names))

    fn = jax.jit(
        shard_map(
            _body,
            mesh=mesh,
            in_specs=specs,
            out_specs=(PartitionSpec("core"),) * len(out_names),
            check_rep=False,
        ),
        keep_unused=True,
    )

    per = {"x1": x1, "x2": x2}
    concat_in = [
        np.concatenate([per[n][b] for b in range(B)], axis=0) for n in in_names
    ]
    concat_zero = [
        np.zeros((B * z.shape[0], *z.shape[1:]), z.dtype) for z in zeros
    ]
    sharding = jax.sharding.NamedSharding(mesh, PartitionSpec("core"))
    dev_args = [jax.device_put(a, sharding) for a in concat_in + concat_zero]

    outs = fn(*dev_args)
    jax.block_until_ready(outs)
    ts = []
    for _ in range(iters):
        t0 = time.perf_counter()
        outs = fn(*dev_args)
        jax.block_until_ready(outs)
        ts.append(time.perf_counter() - t0)
    ts.sort()
    return ts


REPS_LONG = 257


def benchmark(x1: np.ndarray, x2: np.ndarray, iters: int = 10):
    """Per-execution device time via reps-loop slope: two NEFFs (reps=1 and
    reps=REPS_LONG with an on-device For_i around the body); the wall-clock
    difference divided by (REPS_LONG-1) cancels the axon dispatch overhead."""
    nc1 = _build(1)
    t1 = _timed_run(nc1, x1, x2, iters)
    ncN = _build(REPS_LONG)
    tN = _timed_run(ncN, x1, x2, iters)
    per_exec = (tN[0] - t1[0]) / (REPS_LONG - 1)
    return per_exec, t1, tN


# revision 11
# speedup vs baseline: 1.1596x; 1.1596x over previous
"""Local 9x9 correlation (cost volume) kernel for Trainium2.

out[b, di*9+dj, h, w] = (1/C) * sum_c x1[b,c,h,w] * x2pad[b,c,h+di,w+dj]

Strategy: batch-parallel across 8 NeuronCores (1 sample each). On-core, the
PE computes block-correlations with x1 stationary: each matmul packs an
8-row x 16-col block of x1 (HB*MB = 128 lhsT columns, ordered (m, hg)) and
streams the corresponding padded-x2 window of 24 cols x 16 rows in (n, r)
order (384 moving columns). Output partition p = 8m+hg holds output pixel
(h = 8G+hg, w = 16j+m); its 81 correlations live at PSUM cols
(m%4+dj)*16 + hg+di inside a 192-col window shared by its 32-partition
group, so PSUM evacuation extracts the window with plain rectangular
bf16 copies and each 32-row chunk is dumped with a single fully-contiguous
DMA. This streams ~4x fewer PE columns than a per-row banded Gram matrix
and writes 4.7 MB instead of 8.9 MB.
"""

import numpy as np

B, C, H, W = 8, 256, 96, 128
R = 4                 # correlation radius
D = 2 * R + 1         # 9 displacements per axis
HCHUNK = 32
NCHUNK = H // HCHUNK  # 3
HB = 8                # output rows per matmul block
MB = 16               # output cols per matmul block
NG = HCHUNK // HB     # 4 row-groups per chunk
NSTRIP = W // MB      # 8 col-strips
RWIN = HB + 2 * R     # 16 streamed x2 rows per block
NWIN = MB + 2 * R     # 24 streamed x2 cols per block
NCOL = RWIN * NWIN    # 384 PSUM cols per block
GWIN = 192            # PSUM window holding one 32-partition group's outputs
PADW = W + 2 * R      # 136
PADROWS = HCHUNK + 2 * R  # 40

_compiled = None
last_results = None  # BassKernelResults of the most recent run (for profiling)


def _build(reps: int = 1):
    import contextlib

    import concourse.bass as bass  # noqa: F401
    import concourse.tile as tile
    from concourse import bacc, mybir

    nc = bacc.Bacc(
        "TRN2", target_bir_lowering=False, debug=False, num_devices=8
    )
    # x1 arrives host-pre-blocked: flat (k, g, j, m, hg) so each matmul's
    # 128 stationary columns (p = 8m+hg) are contiguous
    x1 = nc.dram_tensor(
        "x1", [C, H * W], mybir.dt.float32, kind="ExternalInput"
    ).ap()
    x2 = nc.dram_tensor("x2", [C, H, W], mybir.dt.float32, kind="ExternalInput").ap()
    dump = nc.dram_tensor(
        "dump", [128, NCHUNK, NG, NSTRIP, GWIN], mybir.dt.bfloat16,
        kind="ExternalOutput",
    ).ap()

    with tile.TileContext(nc) as tc:
        with (
            tc.tile_pool(name="x1p", bufs=2) as x1p,
            tc.tile_pool(name="x2p", bufs=2) as x2p,
            tc.tile_pool(name="stg", bufs=2) as stg,
            tc.tile_pool(name="ps", bufs=8, space="PSUM") as psp,
            tc.For_i(0, reps, 1) if reps > 1 else contextlib.nullcontext(),
        ):
            prev_x2c = None
            for k in range(NCHUNK):
                h0 = k * HCHUNK

                x1c = x1p.tile([128, 2, HCHUNK * W], mybir.dt.bfloat16)
                for cc in range(2):
                    nc.gpsimd.dma_start(
                        out=x1c[:, cc, :],
                        in_=x1[
                            cc * 128 : (cc + 1) * 128,
                            k * HCHUNK * W : (k + 1) * HCHUNK * W,
                        ],
                    )

                # padded x2 slab: local row p corresponds to x2 row h0-R+p
                x2c = x2p.tile([128, 2, PADROWS, PADW], mybir.dt.bfloat16)
                src_r0 = h0 - R
                nc.vector.memset(x2c[:, :, :, 0:R], 0.0)
                nc.vector.memset(x2c[:, :, :, PADW - R : PADW], 0.0)
                if k == 0:
                    # rows [0, R) are above the image: zero them
                    lo = R
                    nc.vector.memset(x2c[:, :, 0:R, 0:PADW], 0.0)
                else:
                    # halo rows [0, 2R) = previous chunk's local rows
                    # [HCHUNK, HCHUNK+2R) - SBUF->SBUF copy, no HBM re-read
                    lo = 2 * R
                    nc.sync.dma_start(
                        out=x2c[:, :, 0 : 2 * R, :],
                        in_=prev_x2c[:, :, HCHUNK : HCHUNK + 2 * R, :],
                    )
                hi = min(PADROWS, H - src_r0)
                if hi < PADROWS:
                    nc.vector.memset(x2c[:, :, hi:PADROWS, 0:PADW], 0.0)
                for cc in range(2):
                    nc.gpsimd.dma_start(
                        out=x2c[:, cc, lo:hi, R : R + W],
                        in_=x2[
                            cc * 128 : (cc + 1) * 128, src_r0 + lo : src_r0 + hi, :
                        ],
                    )
                prev_x2c = x2c

                stage = stg.tile(
                    [128, NG, NSTRIP, GWIN], mybir.dt.bfloat16
                )
                for g in range(NG):
                    for j in range(NSTRIP):
                        psb = psp.tile([128, 512], mybir.dt.float32)
                        ps = psb[:, 0:NCOL]
                        blk = (g * NSTRIP + j) * 128
                        for cc in range(2):
                            # lhsT: pre-blocked x1 (m, hg) -> p = 8m + hg
                            # rhs: x2 window, streamed (n, r) -> col = n*16+r
                            nc.tensor.matmul(
                                out=ps[:, :],
                                lhsT=x1c[:, cc, blk : blk + 128],
                                rhs=x2c[
                                    :, cc, HB * g : HB * g + RWIN,
                                    MB * j : MB * j + NWIN,
                                ].transpose([0, 2, 1]),
                                start=(cc == 0),
                                stop=(cc == 1),
                                skip_group_check=True,
                            )
                        # extract each 32-partition group's contiguous window
                        for s in range(4):
                            dst = stage[32 * s : 32 * (s + 1), g, j, :]
                            src = ps[32 * s : 32 * (s + 1), 64 * s : 64 * s + GWIN]
                            if (j + s) % 2 == 0:
                                nc.vector.tensor_scalar_mul(dst, src, 1.0)
                            else:
                                nc.scalar.mul(dst, src, 1.0)

                nc.sync.dma_start(
                    out=dump[:, k, :, :, :], in_=stage[:, :, :, :]
                )

    nc.compile()
    return nc


_DESKEW_IDX = None


def _deskew_idx():
    global _DESKEW_IDX
    if _DESKEW_IDX is None:
        p = np.arange(128)
        m, hg = p // HB, p % HB
        di = np.arange(D)
        dj = np.arange(D)
        # c2[p, di, dj] = (m%4 + dj)*16 + hg + di
        _DESKEW_IDX = (
            ((m % 4)[:, None, None] + dj[None, None, :]) * RWIN
            + hg[:, None, None]
            + di[None, :, None]
        )
    return _DESKEW_IDX


def _deskew(dump_b: np.ndarray) -> np.ndarray:
    """[128, NCHUNK, NG, NSTRIP, GWIN] bf16 dump -> [81, H, W] fp32."""
    d = np.asarray(dump_b).astype(np.float32) * np.float32(1.0 / C)
    idx = _deskew_idx()
    pidx = np.arange(128)[:, None, None]
    # V[p, di, dj, k, g, j] = d[p, k, g, j, idx[p, di, dj]]
    V = d[pidx, :, :, :, idx]  # [128, 9, 9, 3, 4, 8]
    V = V.reshape(MB, HB, D, D, NCHUNK, NG, NSTRIP)  # [m, hg, di, dj, k, g, j]
    # out[di*9+dj, h=(k,g,hg), w=(j,m)]
    out = V.transpose(2, 3, 4, 5, 1, 6, 0)  # [di, dj, k, g, hg, j, m]
    return np.ascontiguousarray(out.reshape(D * D, H, W))


def _block_x1(x1: np.ndarray) -> np.ndarray:
    """[B?, C, H, W] -> [..., C, H*W] flat (k, g, j, m, hg) blocked order."""
    lead = x1.shape[:-3]
    xb = x1.reshape(*lead, C, NCHUNK, NG, HB, NSTRIP, MB)
    xb = xb.transpose(*range(len(lead)), len(lead), len(lead) + 1,
                      len(lead) + 2, len(lead) + 4, len(lead) + 5,
                      len(lead) + 3)            # [..., c, k, g, j, m, hg]
    return np.ascontiguousarray(xb.reshape(*lead, C, H * W))


def kernel(x1: np.ndarray, x2: np.ndarray) -> np.ndarray:
    global _compiled, last_results
    import os

    os.environ["BASS_NEVER_TRACE"] = "1"
    from concourse.bass_utils import run_bass_kernel_spmd

    x1 = np.ascontiguousarray(np.asarray(x1), dtype=np.float32)
    x2 = np.ascontiguousarray(np.asarray(x2), dtype=np.float32)
    assert x1.shape == (B, C, H, W) and x2.shape == (B, C, H, W)
    x1b = _block_x1(x1)

    if _compiled is None:
        _compiled = _build()
    nc = _compiled

    in_maps = [{"x1": x1b[b], "x2": x2[b]} for b in range(B)]
    res = run_bass_kernel_spmd(nc, in_maps, core_ids=list(range(B)))
    last_results = res

    return np.stack([_deskew(res.results[b]["dump"]) for b in range(B)], axis=0)


def _timed_run(nc, x1, x2, iters):
    import time

    import jax
    from jax.experimental.shard_map import shard_map
    from jax.sharding import Mesh, PartitionSpec

    from concourse import bass2jax, mybir

    bass2jax.install_neuronx_cc_hook()

    partition_name = (
        nc.partition_id_tensor.name if nc.partition_id_tensor else None
    )
    in_names, out_names, out_avals, zeros = [], [], [], []
    for alloc in nc.m.functions[0].allocations:
        if not isinstance(alloc, mybir.MemoryLocationSet):
            continue
        name = alloc.memorylocations[0].name
        if alloc.kind == "ExternalInput":
            if name != partition_name:
                in_names.append(name)
        elif alloc.kind == "ExternalOutput":
            shape = tuple(alloc.tensor_shape)
            dtype = mybir.dt.np(alloc.dtype)
            out_names.append(name)
            out_avals.append(jax.core.ShapedArray(shape, dtype))
            zeros.append(np.zeros(shape, dtype))
    n_params = len(in_names)
    all_names = in_names + out_names
    if partition_name is not None:
        all_names = all_names + [partition_name]

    def _body(*args):
        operands = list(args)
        if partition_name is not None:
            operands.append(bass2jax.partition_id_tensor())
        return tuple(
            bass2jax._bass_exec_p.bind(
                *operands,
                out_avals=tuple(out_avals),
                in_names=tuple(all_names),
                out_names=tuple(out_names),
                lowering_input_output_aliases=(),
                sim_require_finite=True,
                sim_require_nnan=True,
                nc=nc,
            )
        )

    devices = jax.devices()[:B]
    mesh = Mesh(np.asarray(devices), ("core",))
    specs = (PartitionSpec("core"),) * (n_params + len(out_names))

    fn = jax.jit(
        shard_map(
            _body,
            mesh=mesh,
            in_specs=specs,
            out_specs=(PartitionSpec("core"),) * len(out_names),
            check_rep=False,
        ),
        keep_unused=True,
    )

    per = {"x1": x1, "x2": x2}
    concat_in = [
        np.concatenate([per[n][b] for b in range(B)], axis=0) for n in in_names
    ]
    concat_zero = [
        np.zeros((B * z.shape[0], *z.shape[1:]), z.dtype) for z in zeros
    ]
    sharding = jax.sharding.NamedSharding(mesh, PartitionSpec("core"))
    dev_args = [jax.device_put(a, sharding) for a in concat_in + concat_zero]

    outs = fn(*dev_args)
    jax.block_until_ready(outs)
    ts = []
    for _ in range(iters):
        t0 = time.perf_counter()
        outs = fn(*dev_args)
        jax.block_until_ready(outs)
        ts.append(time.perf_counter() - t0)
    ts.sort()
    return ts


REPS_LONG = 257


def benchmark(x1: np.ndarray, x2: np.ndarray, iters: int = 10):
    """Per-execution device time via reps-loop slope: two NEFFs (reps=1 and
    reps=REPS_LONG with an on-device For_i around the body); the wall-clock
    difference divided by (REPS_LONG-1) cancels the axon dispatch overhead."""
    x1 = _block_x1(np.ascontiguousarray(np.asarray(x1), dtype=np.float32))
    nc1 = _build(1)
    t1 = _timed_run(nc1, x1, x2, iters)
    ncN = _build(REPS_LONG)
    tN = _timed_run(ncN, x1, x2, iters)
    per_exec = (tN[0] - t1[0]) / (REPS_LONG - 1)
    return per_exec, t1, tN


# revision 13
# speedup vs baseline: 1.2929x; 1.1149x over previous
"""Local 9x9 correlation (cost volume) kernel for Trainium2.

out[b, di*9+dj, h, w] = (1/C) * sum_c x1[b,c,h,w] * x2pad[b,c,h+di,w+dj]

Strategy: batch-parallel across 8 NeuronCores (1 sample each). On-core, the
PE computes block-correlations with x1 stationary: each matmul packs an
8-row x 16-col block of x1 (HB*MB = 128 lhsT columns, ordered (m, hg)) and
streams the corresponding padded-x2 window of 24 cols x 16 rows in (n, r)
order (384 moving columns). Output partition p = 8m+hg holds output pixel
(h = 8G+hg, w = 16j+m); its 81 correlations live at PSUM cols
(m%4+dj)*16 + hg+di inside a 192-col window shared by its 32-partition
group, so PSUM evacuation extracts the window with plain rectangular
bf16 copies and each 32-row chunk is dumped with a single fully-contiguous
DMA. This streams ~4x fewer PE columns than a per-row banded Gram matrix
and writes 4.7 MB instead of 8.9 MB.
"""

import numpy as np

B, C, H, W = 8, 256, 96, 128
R = 4                 # correlation radius
D = 2 * R + 1         # 9 displacements per axis
HCHUNK = 32
NCHUNK = H // HCHUNK  # 3
HB = 8                # output rows per matmul block
MB = 16               # output cols per matmul block
NG = HCHUNK // HB     # 4 row-groups per chunk
NSTRIP = W // MB      # 8 col-strips
RWIN = HB + 2 * R     # 16 streamed x2 rows per block
NWIN = MB + 2 * R     # 24 streamed x2 cols per block
NCOL = RWIN * NWIN    # 384 PSUM cols per block
GWIN = 192            # PSUM window holding one 32-partition group's outputs
PADW = W + 2 * R      # 136
PADROWS = HCHUNK + 2 * R  # 40

_compiled = None
last_results = None  # BassKernelResults of the most recent run (for profiling)


def _build(reps: int = 1):
    import contextlib

    import concourse.bass as bass  # noqa: F401
    import concourse.tile as tile
    from concourse import bacc, mybir

    nc = bacc.Bacc(
        "TRN2", target_bir_lowering=False, debug=False, num_devices=8
    )
    # x1 arrives host-pre-blocked: flat (k, g, j, m, hg) so each matmul's
    # 128 stationary columns (p = 8m+hg) are contiguous
    x1 = nc.dram_tensor(
        "x1", [C, H * W], mybir.dt.float32, kind="ExternalInput"
    ).ap()
    x2 = nc.dram_tensor("x2", [C, H, W], mybir.dt.float32, kind="ExternalInput").ap()
    dump = nc.dram_tensor(
        "dump", [128, NCHUNK, NG, NSTRIP, GWIN], mybir.dt.bfloat16,
        kind="ExternalOutput",
    ).ap()

    with tile.TileContext(nc) as tc:
        with (
            tc.tile_pool(name="x1p", bufs=2) as x1p,
            tc.tile_pool(name="x2p", bufs=2) as x2p,
            tc.tile_pool(name="stg", bufs=2) as stg,
            tc.tile_pool(name="ps", bufs=8, space="PSUM") as psp,
            tc.For_i(0, reps, 1) if reps > 1 else contextlib.nullcontext(),
        ):
            prev_x2c = None
            for k in range(NCHUNK):
                h0 = k * HCHUNK

                x1c = x1p.tile([128, 2, HCHUNK * W], mybir.dt.bfloat16)
                for cc in range(2):
                    nc.gpsimd.dma_start(
                        out=x1c[:, cc, :],
                        in_=x1[
                            cc * 128 : (cc + 1) * 128,
                            k * HCHUNK * W : (k + 1) * HCHUNK * W,
                        ],
                    )

                # padded x2 slab: local row p corresponds to x2 row h0-R+p
                x2c = x2p.tile([128, 2, PADROWS, PADW], mybir.dt.bfloat16)
                src_r0 = h0 - R
                nc.vector.memset(x2c[:, :, :, 0:R], 0.0)
                nc.vector.memset(x2c[:, :, :, PADW - R : PADW], 0.0)
                if k == 0:
                    # rows [0, R) are above the image: zero them
                    lo = R
                    nc.vector.memset(x2c[:, :, 0:R, 0:PADW], 0.0)
                else:
                    # halo rows [0, 2R) = previous chunk's local rows
                    # [HCHUNK, HCHUNK+2R) - SBUF->SBUF copy, no HBM re-read
                    lo = 2 * R
                    nc.sync.dma_start(
                        out=x2c[:, :, 0 : 2 * R, :],
                        in_=prev_x2c[:, :, HCHUNK : HCHUNK + 2 * R, :],
                    )
                hi = min(PADROWS, H - src_r0)
                if hi < PADROWS:
                    nc.vector.memset(x2c[:, :, hi:PADROWS, 0:PADW], 0.0)
                for cc in range(2):
                    nc.gpsimd.dma_start(
                        out=x2c[:, cc, lo:hi, R : R + W],
                        in_=x2[
                            cc * 128 : (cc + 1) * 128, src_r0 + lo : src_r0 + hi, :
                        ],
                    )
                prev_x2c = x2c

                stage = stg.tile(
                    [128, NG, NSTRIP, RWIN, MB // 4 + D - 1], mybir.dt.bfloat16
                )
                for g in range(NG):
                    for j in range(NSTRIP):
                        psb = psp.tile([128, 512], mybir.dt.float32)
                        ps = psb[:, 0:NCOL]
                        blk = (g * NSTRIP + j) * 128
                        for cc in range(2):
                            # lhsT: pre-blocked x1 (m, hg) -> p = 8m + hg
                            # rhs: x2 window, streamed (r, n) -> col = r*24+n
                            # (contiguous-inner for the PE stream)
                            nc.tensor.matmul(
                                out=ps[:, :],
                                lhsT=x1c[:, cc, blk : blk + 128],
                                rhs=x2c[
                                    :, cc, HB * g : HB * g + RWIN,
                                    MB * j : MB * j + NWIN,
                                ],
                                start=(cc == 0),
                                stop=(cc == 1),
                                skip_group_check=True,
                            )
                        # extract each 32-partition group's (r, n) window
                        ps3 = ps.rearrange("p (r n) -> p r n", n=NWIN)
                        for s in range(4):
                            dst = stage[32 * s : 32 * (s + 1), g, j, :, :]
                            src = ps3[
                                32 * s : 32 * (s + 1), :, 4 * s : 4 * s + 12
                            ]
                            if (j + s) % 2 == 0:
                                nc.vector.tensor_scalar_mul(dst, src, 1.0)
                            else:
                                nc.scalar.mul(dst, src, 1.0)

                nc.sync.dma_start(
                    out=dump[:, k, :, :, :], in_=stage[:, :, :, :]
                )

    nc.compile()
    return nc


_DESKEW_IDX = None


def _deskew_idx():
    global _DESKEW_IDX
    if _DESKEW_IDX is None:
        p = np.arange(128)
        m, hg = p // HB, p % HB
        di = np.arange(D)
        dj = np.arange(D)
        # c2[p, di, dj] = (hg + di)*12 + m%4 + dj
        _DESKEW_IDX = (
            (hg[:, None, None] + di[None, :, None]) * 12
            + (m % 4)[:, None, None]
            + dj[None, None, :]
        )
    return _DESKEW_IDX


def _deskew(dump_b: np.ndarray) -> np.ndarray:
    """[128, NCHUNK, NG, NSTRIP, GWIN] bf16 dump -> [81, H, W] fp32."""
    d = np.asarray(dump_b).astype(np.float32) * np.float32(1.0 / C)
    idx = _deskew_idx()
    pidx = np.arange(128)[:, None, None]
    # V[p, di, dj, k, g, j] = d[p, k, g, j, idx[p, di, dj]]
    V = d[pidx, :, :, :, idx]  # [128, 9, 9, 3, 4, 8]
    V = V.reshape(MB, HB, D, D, NCHUNK, NG, NSTRIP)  # [m, hg, di, dj, k, g, j]
    # out[di*9+dj, h=(k,g,hg), w=(j,m)]
    out = V.transpose(2, 3, 4, 5, 1, 6, 0)  # [di, dj, k, g, hg, j, m]
    return np.ascontiguousarray(out.reshape(D * D, H, W))


def _block_x1(x1: np.ndarray) -> np.ndarray:
    """[B?, C, H, W] -> [..., C, H*W] flat (k, g, j, m, hg) blocked order."""
    lead = x1.shape[:-3]
    xb = x1.reshape(*lead, C, NCHUNK, NG, HB, NSTRIP, MB)
    xb = xb.transpose(*range(len(lead)), len(lead), len(lead) + 1,
                      len(lead) + 2, len(lead) + 4, len(lead) + 5,
                      len(lead) + 3)            # [..., c, k, g, j, m, hg]
    return np.ascontiguousarray(xb.reshape(*lead, C, H * W))


def kernel(x1: np.ndarray, x2: np.ndarray) -> np.ndarray:
    global _compiled, last_results
    import os

    os.environ["BASS_NEVER_TRACE"] = "1"
    from concourse.bass_utils import run_bass_kernel_spmd

    x1 = np.ascontiguousarray(np.asarray(x1), dtype=np.float32)
    x2 = np.ascontiguousarray(np.asarray(x2), dtype=np.float32)
    assert x1.shape == (B, C, H, W) and x2.shape == (B, C, H, W)
    x1b = _block_x1(x1)

    if _compiled is None:
        _compiled = _build()
    nc = _compiled

    in_maps = [{"x1": x1b[b], "x2": x2[b]} for b in range(B)]
    res = run_bass_kernel_spmd(nc, in_maps, core_ids=list(range(B)))
    last_results = res

    return np.stack([_deskew(res.results[b]["dump"]) for b in range(B)], axis=0)


def _timed_run(nc, x1, x2, iters):
    import time

    import jax
    from jax.experimental.shard_map import shard_map
    from jax.sharding import Mesh, PartitionSpec

    from concourse import bass2jax, mybir

    bass2jax.install_neuronx_cc_hook()

    partition_name = (
        nc.partition_id_tensor.name if nc.partition_id_tensor else None
    )
    in_names, out_names, out_avals, zeros = [], [], [], []
    for alloc in nc.m.functions[0].allocations:
        if not isinstance(alloc, mybir.MemoryLocationSet):
            continue
        name = alloc.memorylocations[0].name
        if alloc.kind == "ExternalInput":
            if name != partition_name:
                in_names.append(name)
        elif alloc.kind == "ExternalOutput":
            shape = tuple(alloc.tensor_shape)
            dtype = mybir.dt.np(alloc.dtype)
            out_names.append(name)
            out_avals.append(jax.core.ShapedArray(shape, dtype))
            zeros.append(np.zeros(shape, dtype))
    n_params = len(in_names)
    all_names = in_names + out_names
    if partition_name is not None:
        all_names = all_names + [partition_name]

    def _body(*args):
        operands = list(args)
        if partition_name is not None:
            operands.append(bass2jax.partition_id_tensor())
        return tuple(
            bass2jax._bass_exec_p.bind(
                *operands,
                out_avals=tuple(out_avals),
                in_names=tuple(all_names),
                out_names=tuple(out_names),
                lowering_input_output_aliases=(),
                sim_require_finite=True,
                sim_require_nnan=True,
                nc=nc,
            )
        )

    devices = jax.devices()[:B]
    mesh = Mesh(np.asarray(devices), ("core",))
    specs = (PartitionSpec("core"),) * (n_params + len(out_names))

    fn = jax.jit(
        shard_map(
            _body,
            mesh=mesh,
            in_specs=specs,
            out_specs=(PartitionSpec("core"),) * len(out_names),
            check_rep=False,
        ),
        keep_unused=True,
    )

    per = {"x1": x1, "x2": x2}
    concat_in = [
        np.concatenate([per[n][b] for b in range(B)], axis=0) for n in in_names
    ]
    concat_zero = [
        np.zeros((B * z.shape[0], *z.shape[1:]), z.dtype) for z in zeros
    ]
    sharding = jax.sharding.NamedSharding(mesh, PartitionSpec("core"))
    dev_args = [jax.device_put(a, sharding) for a in concat_in + concat_zero]

    outs = fn(*dev_args)
    jax.block_until_ready(outs)
    ts = []
    for _ in range(iters):
        t0 = time.perf_counter()
        outs = fn(*dev_args)
        jax.block_until_ready(outs)
        ts.append(time.perf_counter() - t0)
    ts.sort()
    return ts


REPS_LONG = 257


def benchmark(x1: np.ndarray, x2: np.ndarray, iters: int = 10):
    """Per-execution device time via reps-loop slope: two NEFFs (reps=1 and
    reps=REPS_LONG with an on-device For_i around the body); the wall-clock
    difference divided by (REPS_LONG-1) cancels the axon dispatch overhead."""
    x1 = _block_x1(np.ascontiguousarray(np.asarray(x1), dtype=np.float32))
    nc1 = _build(1)
    t1 = _timed_run(nc1, x1, x2, iters)
    ncN = _build(REPS_LONG)
    tN = _timed_run(ncN, x1, x2, iters)
    per_exec = (tN[0] - t1[0]) / (REPS_LONG - 1)
    return per_exec, t1, tN


# revision 16
# speedup vs baseline: 1.5976x; 1.2357x over previous
"""Local 9x9 correlation (cost volume) kernel for Trainium2.

out[b, di*9+dj, h, w] = (1/C) * sum_c x1[b,c,h,w] * x2pad[b,c,h+di,w+dj]

Strategy: batch-parallel across 8 NeuronCores (1 sample each). On-core, the
PE computes block-correlations with x1 stationary: each matmul packs an
8-row x 16-col block of x1 (HB*MB = 128 lhsT columns, ordered (m, hg)) and
streams the corresponding padded-x2 window of 24 cols x 16 rows in (n, r)
order (384 moving columns). Output partition p = 8m+hg holds output pixel
(h = 8G+hg, w = 16j+m); its 81 correlations live at PSUM cols
(m%4+dj)*16 + hg+di inside a 192-col window shared by its 32-partition
group, so PSUM evacuation extracts the window with plain rectangular
bf16 copies and each 32-row chunk is dumped with a single fully-contiguous
DMA. This streams ~4x fewer PE columns than a per-row banded Gram matrix
and writes 4.7 MB instead of 8.9 MB.
"""

import numpy as np

B, C, H, W = 8, 256, 96, 128
R = 4                 # correlation radius
D = 2 * R + 1         # 9 displacements per axis
HCHUNK = 32
NCHUNK = H // HCHUNK  # 3
HB = 8                # output rows per matmul block
MB = 16               # output cols per matmul block
NG = HCHUNK // HB     # 4 row-groups per chunk
NSTRIP = W // MB      # 8 col-strips
RWIN = HB + 2 * R     # 16 streamed x2 rows per block
NWIN = MB + 2 * R     # 24 streamed x2 cols per block
NCOL = RWIN * NWIN    # 384 PSUM cols per block
GWIN = 192            # PSUM window holding one 32-partition group's outputs
PADW = W + 2 * R      # 136
PADROWS = HCHUNK + 2 * R  # 40

_compiled = None
last_results = None  # BassKernelResults of the most recent run (for profiling)


def _build(reps: int = 1):
    import contextlib

    import concourse.bass as bass  # noqa: F401
    import concourse.tile as tile
    from concourse import bacc, mybir

    nc = bacc.Bacc(
        "TRN2", target_bir_lowering=False, debug=False, num_devices=8
    )
    # x1 arrives host-pre-blocked: flat (k, g, j, m, hg) so each matmul's
    # 128 stationary columns (p = 8m+hg) are contiguous
    x1 = nc.dram_tensor(
        "x1", [C, H * W], mybir.dt.float32, kind="ExternalInput"
    ).ap()
    x2 = nc.dram_tensor("x2", [C, H, W], mybir.dt.float32, kind="ExternalInput").ap()
    dump = nc.dram_tensor(
        "dump", [128, NCHUNK, NG, NSTRIP, NCOL], mybir.dt.bfloat16,
        kind="ExternalOutput",
    ).ap()

    with tile.TileContext(nc) as tc:
        with (
            tc.tile_pool(name="x1p", bufs=2) as x1p,
            tc.tile_pool(name="x2p", bufs=2) as x2p,
            tc.tile_pool(name="stg", bufs=2) as stg,
            tc.tile_pool(name="ps", bufs=8, space="PSUM") as psp,
            tc.For_i(0, reps, 1) if reps > 1 else contextlib.nullcontext(),
        ):
            prev_x2c = None
            for k in range(NCHUNK):
                h0 = k * HCHUNK

                x1c = x1p.tile([128, 2, HCHUNK * W], mybir.dt.bfloat16)
                for cc in range(2):
                    nc.gpsimd.dma_start(
                        out=x1c[:, cc, :],
                        in_=x1[
                            cc * 128 : (cc + 1) * 128,
                            k * HCHUNK * W : (k + 1) * HCHUNK * W,
                        ],
                    )

                # padded x2 slab: local row p corresponds to x2 row h0-R+p
                x2c = x2p.tile([128, 2, PADROWS, PADW], mybir.dt.bfloat16)
                src_r0 = h0 - R
                nc.vector.memset(x2c[:, :, :, 0:R], 0.0)
                nc.vector.memset(x2c[:, :, :, PADW - R : PADW], 0.0)
                if k == 0:
                    # rows [0, R) are above the image: zero them
                    lo = R
                    nc.vector.memset(x2c[:, :, 0:R, 0:PADW], 0.0)
                else:
                    # halo rows [0, 2R) = previous chunk's local rows
                    # [HCHUNK, HCHUNK+2R) - SBUF->SBUF copy, no HBM re-read
                    lo = 2 * R
                    nc.sync.dma_start(
                        out=x2c[:, :, 0 : 2 * R, :],
                        in_=prev_x2c[:, :, HCHUNK : HCHUNK + 2 * R, :],
                    )
                hi = min(PADROWS, H - src_r0)
                if hi < PADROWS:
                    nc.vector.memset(x2c[:, :, hi:PADROWS, 0:PADW], 0.0)
                for cc in range(2):
                    nc.gpsimd.dma_start(
                        out=x2c[:, cc, lo:hi, R : R + W],
                        in_=x2[
                            cc * 128 : (cc + 1) * 128, src_r0 + lo : src_r0 + hi, :
                        ],
                    )
                prev_x2c = x2c

                stage = stg.tile(
                    [128, NG, NSTRIP, NCOL], mybir.dt.bfloat16
                )
                for g in range(NG):
                    for j in range(NSTRIP):
                        psb = psp.tile([128, 512], mybir.dt.float32)
                        ps = psb[:, 0:NCOL]
                        blk = (g * NSTRIP + j) * 128
                        for cc in range(2):
                            # lhsT: pre-blocked x1 (m, hg) -> p = 8m + hg
                            # rhs: x2 window, streamed (r, n) -> col = r*24+n
                            # (contiguous-inner for the PE stream)
                            nc.tensor.matmul(
                                out=ps[:, :],
                                lhsT=x1c[:, cc, blk : blk + 128],
                                rhs=x2c[
                                    :, cc, HB * g : HB * g + RWIN,
                                    MB * j : MB * j + NWIN,
                                ],
                                start=(cc == 0),
                                stop=(cc == 1),
                                skip_group_check=True,
                            )
                        # evacuate the full band with one full-width copy
                        dst = stage[:, g, j, :]
                        if j % 2 == 0:
                            nc.vector.tensor_scalar_mul(dst, ps, 1.0)
                        else:
                            nc.scalar.mul(dst, ps, 1.0)

                nc.sync.dma_start(
                    out=dump[:, k, :, :, :], in_=stage[:, :, :, :]
                )

    nc.compile()
    return nc


_DESKEW_IDX = None


def _deskew_idx():
    global _DESKEW_IDX
    if _DESKEW_IDX is None:
        p = np.arange(128)
        m, hg = p // HB, p % HB
        di = np.arange(D)
        dj = np.arange(D)
        # c2[p, di, dj] = (hg + di)*NWIN + m + dj
        _DESKEW_IDX = (
            (hg[:, None, None] + di[None, :, None]) * NWIN
            + m[:, None, None]
            + dj[None, None, :]
        )
    return _DESKEW_IDX


def _deskew(dump_b: np.ndarray) -> np.ndarray:
    """[128, NCHUNK, NG, NSTRIP, NCOL] bf16 dump -> [81, H, W] fp32."""
    d = np.asarray(dump_b).astype(np.float32) * np.float32(1.0 / C)
    idx = _deskew_idx()
    pidx = np.arange(128)[:, None, None]
    # V[p, di, dj, k, g, j] = d[p, k, g, j, idx[p, di, dj]]
    V = d[pidx, :, :, :, idx]  # [128, 9, 9, 3, 4, 8]
    V = V.reshape(MB, HB, D, D, NCHUNK, NG, NSTRIP)  # [m, hg, di, dj, k, g, j]
    # out[di*9+dj, h=(k,g,hg), w=(j,m)]
    out = V.transpose(2, 3, 4, 5, 1, 6, 0)  # [di, dj, k, g, hg, j, m]
    return np.ascontiguousarray(out.reshape(D * D, H, W))


def _block_x1(x1: np.ndarray) -> np.ndarray:
    """[B?, C, H, W] -> [..., C, H*W] flat (k, g, j, m, hg) blocked order."""
    lead = x1.shape[:-3]
    xb = x1.reshape(*lead, C, NCHUNK, NG, HB, NSTRIP, MB)
    xb = xb.transpose(*range(len(lead)), len(lead), len(lead) + 1,
                      len(lead) + 2, len(lead) + 4, len(lead) + 5,
                      len(lead) + 3)            # [..., c, k, g, j, m, hg]
    return np.ascontiguousarray(xb.reshape(*lead, C, H * W))


def kernel(x1: np.ndarray, x2: np.ndarray) -> np.ndarray:
    global _compiled, last_results
    import os

    os.environ["BASS_NEVER_TRACE"] = "1"
    from concourse.bass_utils import run_bass_kernel_spmd

    x1 = np.ascontiguousarray(np.asarray(x1), dtype=np.float32)
    x2 = np.ascontiguousarray(np.asarray(x2), dtype=np.float32)
    assert x1.shape == (B, C, H, W) and x2.shape == (B, C, H, W)
    x1b = _block_x1(x1)

    if _compiled is None:
        _compiled = _build()
    nc = _compiled

    in_maps = [{"x1": x1b[b], "x2": x2[b]} for b in range(B)]
    res = run_bass_kernel_spmd(nc, in_maps, core_ids=list(range(B)))
    last_results = res

    return np.stack([_deskew(res.results[b]["dump"]) for b in range(B)], axis=0)


def _timed_run(nc, x1, x2, iters):
    import time

    import jax
    from jax.experimental.shard_map import shard_map
    from jax.sharding import Mesh, PartitionSpec

    from concourse import bass2jax, mybir

    bass2jax.install_neuronx_cc_hook()

    partition_name = (
        nc.partition_id_tensor.name if nc.partition_id_tensor else None
    )
    in_names, out_names, out_avals, zeros = [], [], [], []
    for alloc in nc.m.functions[0].allocations:
        if not isinstance(alloc, mybir.MemoryLocationSet):
            continue
        name = alloc.memorylocations[0].name
        if alloc.kind == "ExternalInput":
            if name != partition_name:
                in_names.append(name)
        elif alloc.kind == "ExternalOutput":
            shape = tuple(alloc.tensor_shape)
            dtype = mybir.dt.np(alloc.dtype)
            out_names.append(name)
            out_avals.append(jax.core.ShapedArray(shape, dtype))
            zeros.append(np.zeros(shape, dtype))
    n_params = len(in_names)
    all_names = in_names + out_names
    if partition_name is not None:
        all_names = all_names + [partition_name]

    def _body(*args):
        operands = list(args)
        if partition_name is not None:
            operands.append(bass2jax.partition_id_tensor())
        return tuple(
            bass2jax._bass_exec_p.bind(
                *operands,
                out_avals=tuple(out_avals),
                in_names=tuple(all_names),
                out_names=tuple(out_names),
                lowering_input_output_aliases=(),
                sim_require_finite=True,
                sim_require_nnan=True,
                nc=nc,
            )
        )

    devices = jax.devices()[:B]
    mesh = Mesh(np.asarray(devices), ("core",))
    specs = (PartitionSpec("core"),) * (n_params + len(out_names))

    fn = jax.jit(
        shard_map(
            _body,
            mesh=mesh,
            in_specs=specs,
            out_specs=(PartitionSpec("core"),) * len(out_names),
            check_rep=False,
        ),
        keep_unused=True,
    )

    per = {"x1": x1, "x2": x2}
    concat_in = [
        np.concatenate([per[n][b] for b in range(B)], axis=0) for n in in_names
    ]
    concat_zero = [
        np.zeros((B * z.shape[0], *z.shape[1:]), z.dtype) for z in zeros
    ]
    sharding = jax.sharding.NamedSharding(mesh, PartitionSpec("core"))
    dev_args = [jax.device_put(a, sharding) for a in concat_in + concat_zero]

    outs = fn(*dev_args)
    jax.block_until_ready(outs)
    ts = []
    for _ in range(iters):
        t0 = time.perf_counter()
        outs = fn(*dev_args)
        jax.block_until_ready(outs)
        ts.append(time.perf_counter() - t0)
    ts.sort()
    return ts


REPS_LONG = 257


def benchmark(x1: np.ndarray, x2: np.ndarray, iters: int = 10):
    """Per-execution device time via reps-loop slope: two NEFFs (reps=1 and
    reps=REPS_LONG with an on-device For_i around the body); the wall-clock
    difference divided by (REPS_LONG-1) cancels the axon dispatch overhead."""
    x1 = _block_x1(np.ascontiguousarray(np.asarray(x1), dtype=np.float32))
    nc1 = _build(1)
    t1 = _timed_run(nc1, x1, x2, iters)
    ncN = _build(REPS_LONG)
    tN = _timed_run(ncN, x1, x2, iters)
    per_exec = (tN[0] - t1[0]) / (REPS_LONG - 1)
    return per_exec, t1, tN


# revision 19
# speedup vs baseline: 1.6233x; 1.0161x over previous
"""Local 9x9 correlation (cost volume) kernel for Trainium2.

out[b, di*9+dj, h, w] = (1/C) * sum_c x1[b,c,h,w] * x2pad[b,c,h+di,w+dj]

Strategy: batch-parallel across 8 NeuronCores (1 sample each). On-core, the
PE computes block-correlations with x1 stationary: each matmul packs an
8-row x 16-col block of x1 (HB*MB = 128 lhsT columns, ordered (m, hg)) and
streams the corresponding padded-x2 window of 24 cols x 16 rows in (n, r)
order (384 moving columns). Output partition p = 8m+hg holds output pixel
(h = 8G+hg, w = 16j+m); its 81 correlations live at PSUM cols
(m%4+dj)*16 + hg+di inside a 192-col window shared by its 32-partition
group, so PSUM evacuation extracts the window with plain rectangular
bf16 copies and each 32-row chunk is dumped with a single fully-contiguous
DMA. This streams ~4x fewer PE columns than a per-row banded Gram matrix
and writes 4.7 MB instead of 8.9 MB.
"""

import numpy as np

B, C, H, W = 8, 256, 96, 128
R = 4                 # correlation radius
D = 2 * R + 1         # 9 displacements per axis
HCHUNK = 32
NCHUNK = H // HCHUNK  # 3
HB = 8                # output rows per matmul block
MB = 16               # output cols per matmul block
NG = HCHUNK // HB     # 4 row-groups per chunk
NSTRIP = W // MB      # 8 col-strips
RWIN = HB + 2 * R     # 16 streamed x2 rows per block
NWIN = MB + 2 * R     # 24 streamed x2 cols per block
NCOL = RWIN * NWIN    # 384 PSUM cols per block
GWIN = 192            # PSUM window holding one 32-partition group's outputs
PADW = W + 2 * R      # 136
PADROWS = HCHUNK + 2 * R  # 40

_compiled = None
last_results = None  # BassKernelResults of the most recent run (for profiling)


def _build(reps: int = 1):
    import contextlib

    import concourse.bass as bass  # noqa: F401
    import concourse.tile as tile
    from concourse import bacc, mybir

    nc = bacc.Bacc(
        "TRN2", target_bir_lowering=False, debug=False, num_devices=8
    )
    # x1 arrives host-pre-blocked: flat (k, g, j, m, hg) so each matmul's
    # 128 stationary columns (p = 8m+hg) are contiguous
    x1 = nc.dram_tensor(
        "x1", [C, H * W], mybir.dt.float32, kind="ExternalInput"
    ).ap()
    x2 = nc.dram_tensor("x2", [C, H, W], mybir.dt.float32, kind="ExternalInput").ap()
    dump = nc.dram_tensor(
        "dump", [128, NCHUNK, NG, NSTRIP, GWIN], mybir.dt.bfloat16,
        kind="ExternalOutput",
    ).ap()

    with tile.TileContext(nc) as tc:
        with (
            tc.tile_pool(name="x1p", bufs=2) as x1p,
            tc.tile_pool(name="x2p", bufs=2) as x2p,
            tc.tile_pool(name="stg", bufs=2) as stg,
            tc.tile_pool(name="st2", bufs=2) as st2,
            tc.tile_pool(name="ps", bufs=8, space="PSUM") as psp,
            tc.For_i(0, reps, 1) if reps > 1 else contextlib.nullcontext(),
        ):
            prev_x2c = None
            for k in range(NCHUNK):
                h0 = k * HCHUNK

                x1c = x1p.tile([128, 2, HCHUNK * W], mybir.dt.bfloat16)
                for cc in range(2):
                    nc.gpsimd.dma_start(
                        out=x1c[:, cc, :],
                        in_=x1[
                            cc * 128 : (cc + 1) * 128,
                            k * HCHUNK * W : (k + 1) * HCHUNK * W,
                        ],
                    )

                # padded x2 slab: local row p corresponds to x2 row h0-R+p
                x2c = x2p.tile([128, 2, PADROWS, PADW], mybir.dt.bfloat16)
                src_r0 = h0 - R
                nc.vector.memset(x2c[:, :, :, 0:R], 0.0)
                nc.vector.memset(x2c[:, :, :, PADW - R : PADW], 0.0)
                if k == 0:
                    # rows [0, R) are above the image: zero them
                    lo = R
                    nc.vector.memset(x2c[:, :, 0:R, 0:PADW], 0.0)
                else:
                    # halo rows [0, 2R) = previous chunk's local rows
                    # [HCHUNK, HCHUNK+2R) - SBUF->SBUF copy, no HBM re-read
                    lo = 2 * R
                    nc.sync.dma_start(
                        out=x2c[:, :, 0 : 2 * R, :],
                        in_=prev_x2c[:, :, HCHUNK : HCHUNK + 2 * R, :],
                    )
                hi = min(PADROWS, H - src_r0)
                if hi < PADROWS:
                    nc.vector.memset(x2c[:, :, hi:PADROWS, 0:PADW], 0.0)
                for cc in range(2):
                    nc.gpsimd.dma_start(
                        out=x2c[:, cc, lo:hi, R : R + W],
                        in_=x2[
                            cc * 128 : (cc + 1) * 128, src_r0 + lo : src_r0 + hi, :
                        ],
                    )
                prev_x2c = x2c

                raw = stg.tile([128, NG, NSTRIP, NCOL], mybir.dt.bfloat16)
                stage2 = st2.tile([128, NG, NSTRIP, GWIN], mybir.dt.bfloat16)
                for g in range(NG):
                    for j in range(NSTRIP):
                        psb = psp.tile([128, 512], mybir.dt.float32)
                        ps = psb[:, 0:NCOL]
                        blk = (g * NSTRIP + j) * 128
                        for cc in range(2):
                            # lhsT: pre-blocked x1 (m, hg) -> p = 8m + hg
                            # rhs: x2 window, streamed (r, n) -> col = r*24+n
                            # (contiguous-inner for the PE stream)
                            nc.tensor.matmul(
                                out=ps[:, :],
                                lhsT=x1c[:, cc, blk : blk + 128],
                                rhs=x2c[
                                    :, cc, HB * g : HB * g + RWIN,
                                    MB * j : MB * j + NWIN,
                                ],
                                start=(cc == 0),
                                stop=(cc == 1),
                                skip_group_check=True,
                            )
                        # evacuate the full band with one full-width copy,
                        # transposing to (n, r) so each 32-partition group's
                        # window becomes one contiguous 192-col run
                        src = ps.rearrange("p (r n) -> p n r", n=NWIN)
                        dst = raw[:, g, j, :].rearrange(
                            "p (n r) -> p n r", r=RWIN
                        )
                        if j % 2 == 0:
                            nc.vector.tensor_scalar_mul(dst, src, 1.0)
                        else:
                            nc.scalar.mul(dst, src, 1.0)

                # SBUF->SBUF compaction: each 32-partition group keeps only
                # its contiguous 192-col window of the 384-col (n, r) band
                for s in range(4):
                    nc.sync.dma_start(
                        out=stage2[32 * s : 32 * (s + 1), :, :, :],
                        in_=raw[32 * s : 32 * (s + 1), :, :, 64 * s : 64 * s + GWIN],
                    )
                nc.sync.dma_start(
                    out=dump[:, k, :, :, :], in_=stage2[:, :, :, :]
                )

    nc.compile()
    return nc


_DESKEW_IDX = None


def _deskew_idx():
    global _DESKEW_IDX
    if _DESKEW_IDX is None:
        p = np.arange(128)
        m, hg = p // HB, p % HB
        di = np.arange(D)
        dj = np.arange(D)
        # c2[p, di, dj] = (m%4 + dj)*RWIN + hg + di
        _DESKEW_IDX = (
            ((m % 4)[:, None, None] + dj[None, None, :]) * RWIN
            + hg[:, None, None]
            + di[None, :, None]
        )
    return _DESKEW_IDX


def _deskew(dump_b: np.ndarray) -> np.ndarray:
    """[128, NCHUNK, NG, NSTRIP, GWIN] bf16 dump -> [81, H, W] fp32."""
    d = np.asarray(dump_b).astype(np.float32) * np.float32(1.0 / C)
    idx = _deskew_idx()
    pidx = np.arange(128)[:, None, None]
    # V[p, di, dj, k, g, j] = d[p, k, g, j, idx[p, di, dj]]
    V = d[pidx, :, :, :, idx]  # [128, 9, 9, 3, 4, 8]
    V = V.reshape(MB, HB, D, D, NCHUNK, NG, NSTRIP)  # [m, hg, di, dj, k, g, j]
    # out[di*9+dj, h=(k,g,hg), w=(j,m)]
    out = V.transpose(2, 3, 4, 5, 1, 6, 0)  # [di, dj, k, g, hg, j, m]
    return np.ascontiguousarray(out.reshape(D * D, H, W))


def _block_x1(x1: np.ndarray) -> np.ndarray:
    """[B?, C, H, W] -> [..., C, H*W] flat (k, g, j, m, hg) blocked order."""
    lead = x1.shape[:-3]
    xb = x1.reshape(*lead, C, NCHUNK, NG, HB, NSTRIP, MB)
    xb = xb.transpose(*range(len(lead)), len(lead), len(lead) + 1,
                      len(lead) + 2, len(lead) + 4, len(lead) + 5,
                      len(lead) + 3)            # [..., c, k, g, j, m, hg]
    return np.ascontiguousarray(xb.reshape(*lead, C, H * W))


def kernel(x1: np.ndarray, x2: np.ndarray) -> np.ndarray:
    global _compiled, last_results
    import os

    os.environ["BASS_NEVER_TRACE"] = "1"
    from concourse.bass_utils import run_bass_kernel_spmd

    x1 = np.ascontiguousarray(np.asarray(x1), dtype=np.float32)
    x2 = np.ascontiguousarray(np.asarray(x2), dtype=np.float32)
    assert x1.shape == (B, C, H, W) and x2.shape == (B, C, H, W)
    x1b = _block_x1(x1)

    if _compiled is None:
        _compiled = _build()
    nc = _compiled

    in_maps = [{"x1": x1b[b], "x2": x2[b]} for b in range(B)]
    res = run_bass_kernel_spmd(nc, in_maps, core_ids=list(range(B)))
    last_results = res

    return np.stack([_deskew(res.results[b]["dump"]) for b in range(B)], axis=0)


def _timed_run(nc, x1, x2, iters):
    import time

    import jax
    from jax.experimental.shard_map import shard_map
    from jax.sharding import Mesh, PartitionSpec

    from concourse import bass2jax, mybir

    bass2jax.install_neuronx_cc_hook()

    partition_name = (
        nc.partition_id_tensor.name if nc.partition_id_tensor else None
    )
    in_names, out_names, out_avals, zeros = [], [], [], []
    for alloc in nc.m.functions[0].allocations:
        if not isinstance(alloc, mybir.MemoryLocationSet):
            continue
        name = alloc.memorylocations[0].name
        if alloc.kind == "ExternalInput":
            if name != partition_name:
                in_names.append(name)
        elif alloc.kind == "ExternalOutput":
            shape = tuple(alloc.tensor_shape)
            dtype = mybir.dt.np(alloc.dtype)
            out_names.append(name)
            out_avals.append(jax.core.ShapedArray(shape, dtype))
            zeros.append(np.zeros(shape, dtype))
    n_params = len(in_names)
    all_names = in_names + out_names
    if partition_name is not None:
        all_names = all_names + [partition_name]

    def _body(*args):
        operands = list(args)
        if partition_name is not None:
            operands.append(bass2jax.partition_id_tensor())
        return tuple(
            bass2jax._bass_exec_p.bind(
                *operands,
                out_avals=tuple(out_avals),
                in_names=tuple(all_names),
                out_names=tuple(out_names),
                lowering_input_output_aliases=(),
                sim_require_finite=True,
                sim_require_nnan=True,
                nc=nc,
            )
        )

    devices = jax.devices()[:B]
    mesh = Mesh(np.asarray(devices), ("core",))
    specs = (PartitionSpec("core"),) * (n_params + len(out_names))

    fn = jax.jit(
        shard_map(
            _body,
            mesh=mesh,
            in_specs=specs,
            out_specs=(PartitionSpec("core"),) * len(out_names),
            check_rep=False,
        ),
        keep_unused=True,
    )

    per = {"x1": x1, "x2": x2}
    concat_in = [
        np.concatenate([per[n][b] for b in range(B)], axis=0) for n in in_names
    ]
    concat_zero = [
        np.zeros((B * z.shape[0], *z.shape[1:]), z.dtype) for z in zeros
    ]
    sharding = jax.sharding.NamedSharding(mesh, PartitionSpec("core"))
    dev_args = [jax.device_put(a, sharding) for a in concat_in + concat_zero]

    outs = fn(*dev_args)
    jax.block_until_ready(outs)
    ts = []
    for _ in range(iters):
        t0 = time.perf_counter()
        outs = fn(*dev_args)
        jax.block_until_ready(outs)
        ts.append(time.perf_counter() - t0)
    ts.sort()
    return ts


REPS_LONG = 257


def benchmark(x1: np.ndarray, x2: np.ndarray, iters: int = 10):
    """Per-execution device time via reps-loop slope: two NEFFs (reps=1 and
    reps=REPS_LONG with an on-device For_i around the body); the wall-clock
    difference divided by (REPS_LONG-1) cancels the axon dispatch overhead."""
    x1 = _block_x1(np.ascontiguousarray(np.asarray(x1), dtype=np.float32))
    nc1 = _build(1)
    t1 = _timed_run(nc1, x1, x2, iters)
    ncN = _build(REPS_LONG)
    tN = _timed_run(ncN, x1, x2, iters)
    per_exec = (tN[0] - t1[0]) / (REPS_LONG - 1)
    return per_exec, t1, tN
